# revision 13
# baseline (speedup 1.0000x reference)
"""AdaptiveEdgeSmoothing Trainium2 kernel (v2).

Reference semantics (per sample, 1024x1024 f32 image):
    edges     = |conv3x3(mask, LAPLACIAN)|          (SAME zero pad)
    edge_mask = edges > 0.5*edge_sensitivity
    sm        = mask*(1-bf) + box5(mask)/25*bf,  bf = blur_strength/3
    result    = where(edge_mask, sm, mask)
    out       = (result > final_threshold).astype(f32)

Strategy: B=16 samples sharded 2-per-core across 8 NeuronCores (pure data
parallel).  Per core, 17 row-tiles: 8 tiles of 124 rows per image plus ONE
merged tile carrying both images' last 32 rows (img0 at partitions 0..34,
img1 at 64..98, so every compute operand keeps the required 32-partition
quadrant alignment).  A tile's block holds rows s..s+126 at partitions
0..126 and the 2-row top halo parked at partitions 126..128; the halos of
all 7 mid tiles of an image arrive in ONE strided SWDGE DMA.

Per tile the TensorE computes, via column-shifted rhs views of the
zero-margined block and a DVE-precomputed u3 = x<<1 + x>>1:
    PSUM1 = 9x - box3(x)              (w3b@x + w3a@u3; the Laplacian)
    PSUM2 = x - sm = bf*x - bf/25*box5(x)
                                      (w5a@{x<<2,x>>2,u3} + w5b@x)
The elementwise tail is 2 ops via a registered custom DVE instruction:
    L  = select(PSUM1^2 > thr^2, PSUM2, 0)     (EDGE_GATE_ANT, DVE)
    o  = (x - ft) > L   -> uint8               (scalar_tensor_tensor, Pool)
which is exactly (result > ft): no edge -> x>ft; edge -> x-ft > x-sm <=>
sm>ft.  Output stores ride the otherwise idle sync-engine HWDGE queue so
they never queue ahead of input SWDGE traffic.
"""

import sys

if '/opt/trn_rl_repo' not in sys.path:
    sys.path.insert(0, '/opt/trn_rl_repo')

import numpy as np

import concourse.bass as bass
import concourse.bacc as bacc
import concourse.bass_utils as bass_utils
import concourse.mybir as mybir
from concourse.tile import TileContext, add_dep_helper
from concourse.bass_utils import run_bass_kernel_spmd
from concourse import dve_ops as _dve_ops
from concourse.dve_spec import Spec, Src0, Src1, C0, Zero, select, sq

# Enable walrus's LDWEIGHTS optimization for this kernel's compile:
# consecutive matmuls sharing a stationary operand skip redundant weight
# loads.  (The flag is hardcoded off in bir_verify_and_optimise.)
if not getattr(bass_utils, "_ldw_opt_patched", False):
    _orig_run_command = bass_utils.run_command

    def _run_command_ldw(argv, **kwargs):
        if isinstance(argv, list):
            argv = ["--enable-ldw-opt=true" if a == "--enable-ldw-opt=false"
                    else a for a in argv]
        return _orig_run_command(argv, **kwargs)

    bass_utils.run_command = _run_command_ldw
    bass_utils._ldw_opt_patched = True

# --- custom DVE op: L = select(in0 > s0, in1, 0) ----------------------------
# (in0 = ACT-squared laplacian from SBUF; in1 = x-sm from PSUM.  Only one
# non-scalar input may live in PSUM per instruction.)
EDGE_GATE = _dve_ops.DveOp(
    "EDGE_GATE_ANT",
    Spec(
        body=select(Src0 > C0, Src1, Zero),
        reference=lambda in0, in1, s0, s1, imm2: np.where(
            in0.astype(np.float32) > s0, in1, 0.0
        ).astype(np.float32),
    ),
    subdim=False,
    uops_sha={"v3": "e54edd49cbbf4900", "v4": "1a8a6c5fc1b3b863"},
)
if EDGE_GATE.name not in _dve_ops._SUB_OPCODE_FOR_NAME:
    _dve_ops.OPS.append(EDGE_GATE)
    _dve_ops._SUB_OPCODE_FOR_NAME[EDGE_GATE.name] = (
        max(_dve_ops._SUB_OPCODE_FOR_NAME.values()) + 1
    )
    _dve_ops.CUSTOM_DVE_SPECS[EDGE_GATE.name] = EDGE_GATE.spec
    assert _dve_ops._SUB_OPCODE_FOR_NAME[EDGE_GATE.name] < 0x20

H = W = 1024
N_CORES = 8
IMGS_PER_CORE = 2
F32 = mybir.dt.float32
F32R = mybir.dt.float32r
U8 = mybir.dt.uint8
XP = 1028  # padded block pitch (2-col zero margins each side)

# tiles 0..16: (cls, img, s).  Weight classes:
#   cls 0/1: img0 t0 / img0 mid;  cls 2/3: img1 t0 / img1 mid;
#   cls 4: merged last-32-rows tile for both images.
# Block layout: partition k = row s+k for k<126; partitions 126,127 park
# rows s-2, s-1 (mid tiles only).  Out rows: psum partition m = row s+m.
TILES = []
for img in range(IMGS_PER_CORE):
    for t in range(8):
        TILES.append((img * 2 + (0 if t == 0 else 1), img, 124 * t))
TILES.append((4, 0, 992))  # merged: both images' rows 992..1024
N_TILES = len(TILES)  # 17

# per cls: (k_tot, nout)
CLS_GEOM = {0: (126, 124), 1: (128, 124), 2: (126, 124), 3: (128, 124),
            4: (98, 96)}
CLS_KIND = {0: 0, 1: 1, 2: 0, 3: 1, 4: 2}  # band-template kind


def _bands(kind):
    """Banded [128,128] masks b3 (|d|<=1), b5 (|d|<=2), ident (d==0) with
    d = row(k) - outrow(m) in tile-relative coords; absent rows clipped."""
    rows = np.full(128, 10 ** 6)
    outr = np.full(128, -10 ** 6)
    blk_k = np.zeros(128, np.int32)
    blk_m = np.zeros(128, np.int32)
    if kind in (0, 1):
        for k in range(126):
            rows[k] = k
        if kind == 1:
            rows[126] = -2
            rows[127] = -1
        for m in range(124):
            outr[m] = m
    else:
        for b in range(2):
            base = 64 * b
            for i in range(32):
                rows[base + i] = 2 + i      # rows 992..1024 (990-relative)
                outr[base + i] = 2 + i
                blk_k[base + i] = b
                blk_m[base + i] = b
            rows[base + 32] = 0             # row 990
            rows[base + 33] = 1             # row 991
            blk_k[base + 32] = blk_k[base + 33] = b
    b3 = np.zeros((128, 128), np.float32)
    b5 = np.zeros((128, 128), np.float32)
    idm = np.zeros((128, 128), np.float32)
    for m in range(128):
        if outr[m] < -1000:
            continue
        for k in range(128):
            if rows[k] > 1000 or blk_k[k] != blk_m[m]:
                continue
            d = rows[k] - outr[m]
            if abs(d) <= 1:
                b3[k, m] = 1.0
            if abs(d) <= 2:
                b5[k, m] = 1.0
            if d == 0:
                idm[k, m] = 1.0
    return b3, b5, idm


_BANDS = {kind: _bands(kind) for kind in range(3)}

_compiled = None
last_results = None


def _build():
    nc = bacc.Bacc("TRN2", target_bir_lowering=False, debug=False,
                   num_devices=N_CORES)
    x = nc.dram_tensor("x", [IMGS_PER_CORE, H, W], F32R,
                       kind="ExternalInput")
    # weights: per cls (5) x {w3a, w3b, w5a, w5b} -> [128, 20*128]
    wp = nc.dram_tensor("wp", [128, 5 * 4 * 128], F32R,
                        kind="ExternalInput").ap()
    thr2 = nc.dram_tensor("thr2", [IMGS_PER_CORE + 1, 128, 1], F32,
                          kind="ExternalInput").ap()
    ftd = nc.dram_tensor("ftd", [IMGS_PER_CORE + 1, 128, 1], F32,
                         kind="ExternalInput").ap()
    y = nc.dram_tensor("out", [IMGS_PER_CORE, H, W], U8,
                       kind="ExternalOutput")

    with TileContext(nc) as tc:
        with (
            tc.tile_pool(name="wpool", bufs=1) as wpool,
            tc.tile_pool(name="spool", bufs=1) as spool,
            tc.tile_pool(name="xpool", bufs=1) as xpool,
            tc.tile_pool(name="p1pool", bufs=2, space="PSUM") as p1pool,
            tc.tile_pool(name="p2pool", bufs=2, space="PSUM") as p2pool,
            tc.tile_pool(name="upool", bufs=3) as upool,
            tc.tile_pool(name="sqpool", bufs=3) as sqpool,
            tc.tile_pool(name="xfpool", bufs=3) as xfpool,
            tc.tile_pool(name="lpool", bufs=3) as lpool,
            tc.tile_pool(name="opool", bufs=4) as opool,
        ):
            # --- one-time loads: weights + per-image scalars -------------
            wall = wpool.tile([128, 5 * 4 * 128], F32R, tag="wall")
            # img0's weights (cls 0,1) land first
            nc.scalar.dma_start(out=wall[:, 0:1024], in_=wp[:, 0:1024])
            nc.scalar.dma_start(out=wall[:, 1024:2048], in_=wp[:, 1024:2048])
            nc.scalar.dma_start(out=wall[:, 2048:2560], in_=wp[:, 2048:2560])

            def w_ap(cls, j):
                b = (cls * 4 + j) * 128
                return wall[:, b:b + 128]

            sc_t = []
            for i in range(IMGS_PER_CORE + 1):
                t2 = spool.tile([128, 1], F32, tag=f"t2_{i}")
                f = spool.tile([128, 1], F32, tag=f"ft_{i}")
                nc.scalar.dma_start(out=t2[:], in_=thr2[i])
                nc.scalar.dma_start(out=f[:], in_=ftd[i])
                sc_t.append((t2, f))

            # --- x blocks: one big SBUF slab, 17 blocks of pitch XP ------
            xbig = xpool.tile([128, N_TILES * XP], F32R, tag="xbig")
            x3 = xbig[:, :].rearrange("p (t c) -> p t c", c=XP)
            # zero the 2-col margins of every block (2 strided memsets) and
            # the merged block's dead partition range 32..64 (u3 reads it).
            nc.vector.memset(x3[:, :, 0:2].bitcast(F32), 0)
            nc.vector.memset(x3[:, :, 1026:1028].bitcast(F32), 0)
            nc.gpsimd.memset(x3[32:64, 16, :].bitcast(F32), 0)

            def emit_load(j, gate=None):
                cls, img, s = TILES[j]
                if cls == 4:
                    ld = nc.gpsimd.dma_start(
                        out=x3[0:32, j, 2:1026],
                        in_=x.ap()[0, 992:1024, :])
                    nc.gpsimd.dma_start(
                        out=x3[32:34, j, 2:1026],
                        in_=x.ap()[0, 990:992, :])
                    nc.gpsimd.dma_start(
                        out=x3[64:96, j, 2:1026],
                        in_=x.ap()[1, 992:1024, :])
                    nc.gpsimd.dma_start(
                        out=x3[96:98, j, 2:1026],
                        in_=x.ap()[1, 990:992, :])
                else:
                    ld = nc.gpsimd.dma_start(
                        out=x3[0:126, j, 2:1026],
                        in_=x.ap()[img, s:s + 126, :])
                if gate is not None:
                    add_dep_helper(ld.ins, gate.ins, reason="stagger")

            def emit_halo(img, gate=None):
                # one DMA: rows {124t-2, 124t-1} for t=1..7 -> partitions
                # 126..128 of blocks img*8+1 .. img*8+8
                j0 = img * 8 + 1
                ld = nc.gpsimd.dma_start(
                    out=x3[126:128, j0:j0 + 7, 2:1026],
                    in_=bass.AP(x, img * H * W + 122 * W,
                                [[W, 2], [124 * W, 7], [1, W]]))
                if gate is not None:
                    add_dep_helper(ld.ins, gate.ins, reason="stagger")

            PREFETCH_AHEAD = 3
            for j in range(PREFETCH_AHEAD):
                emit_load(j)
            emit_halo(0)

            # --- main loop ----------------------------------------------
            for j in range(N_TILES):
                cls, img, s = TILES[j]
                k_tot, nout = CLS_GEOM[cls]
                sci = 2 if cls == 4 else img
                t2_ap, ft_ap = sc_t[sci]
                xt = x3[:, j, :]

                u3 = upool.tile([128, 1024], F32R, tag="u3")
                nc.vector.tensor_tensor(
                    u3[0:k_tot, :],
                    xt[0:k_tot, 1:1025].bitcast(F32),
                    xt[0:k_tot, 3:1027].bitcast(F32),
                    mybir.AluOpType.add)

                p1 = p1pool.tile([128, 1024], F32, tag="p1")
                p2 = p2pool.tile([128, 1024], F32, tag="p2")
                # (psum, weight, col-shift (None = u3 rhs), start, stop)
                passes = [
                    (p1, 1, 0, True, False),      # w3b @ x
                    (p1, 0, None, False, True),   # w3a @ u3
                    (p2, 2, -2, True, False),     # w5a @ x<<2
                    (p2, 2, 2, False, False),     # w5a @ x>>2
                    (p2, 2, None, False, False),  # w5a @ u3
                    (p2, 3, 0, False, True),      # w5b @ x
                ]
                first_mm = None
                for ps, wj, sh, st, sp in passes:
                    for c in (0, 512):
                        if sh is None:
                            rhs = u3[0:k_tot, c:c + 512]
                        else:
                            rhs = xt[0:k_tot, 2 + sh + c:2 + sh + c + 512]
                        mm = nc.tensor.matmul(
                            ps[0:nout, c:c + 512],
                            w_ap(cls, wj)[0:k_tot, 0:nout],
                            rhs, start=st, stop=sp)
                        if first_mm is None:
                            first_mm = mm

                # sq = p1^2 (ACT, psum -> sbuf); L = select(sq > thr^2, p2, 0)
                sq_t = sqpool.tile([128, 1024], F32, tag="sq")
                nc.scalar.activation(sq_t[0:nout, :], p1[0:nout, :],
                                     mybir.ActivationFunctionType.Square)
                lt = lpool.tile([128, 1024], F32, tag="L")
                nc.vector._custom_dve(
                    EDGE_GATE, out=lt[0:nout, :], in0=sq_t[0:nout, :],
                    in1=p2[0:nout, :], s0=t2_ap[0:nout, :])
                # o = (x - ft) > L  (uint8, DVE)
                o_t = opool.tile([128, 1024], U8, tag="o")
                nc.vector.scalar_tensor_tensor(
                    o_t[0:nout, :],
                    xt[0:nout, 2:1026].bitcast(F32),
                    ft_ap[0:nout, :],
                    lt[0:nout, :],
                    mybir.AluOpType.subtract,
                    mybir.AluOpType.is_gt)
                if cls == 4:
                    nc.sync.dma_start(out=y.ap()[0, 992:1024, :],
                                      in_=o_t[0:32, :])
                    nc.sync.dma_start(out=y.ap()[1, 992:1024, :],
                                      in_=o_t[64:96, :])
                else:
                    nc.sync.dma_start(out=y.ap()[img, s:s + nout, :],
                                      in_=o_t[0:nout, :])

                if j == 4:
                    emit_halo(1, first_mm)
                nxt = j + PREFETCH_AHEAD
                if nxt < N_TILES:
                    emit_load(nxt, first_mm)
    nc.compile()
    return nc


def _in_maps(mask, blur_strength, edge_sensitivity, final_threshold):
    mask = np.ascontiguousarray(mask.reshape(16, H, W), np.float32)
    bs = np.asarray(blur_strength, np.float32).reshape(16)
    es = np.asarray(edge_sensitivity, np.float32).reshape(16)
    fts = np.asarray(final_threshold, np.float32).reshape(16)

    maps = []
    for c in range(N_CORES):
        ii = [2 * c, 2 * c + 1]
        bf = [float(bs[i]) / 3.0 for i in ii]
        wp = np.zeros((5, 4, 128, 128), np.float32)
        for cls in range(5):
            kind = CLS_KIND[cls]
            b3, b5, idm = _BANDS[kind]
            wp[cls, 0] = -b3
            wp[cls, 1] = 9.0 * idm - b3
            if cls == 4:
                for bi, k0 in enumerate((0, 64)):
                    k1 = k0 + 34
                    wp[cls, 2][k0:k1] = -(bf[bi] / 25.0) * b5[k0:k1]
                    wp[cls, 3][k0:k1] = (bf[bi] * idm[k0:k1]
                                         - (bf[bi] / 25.0) * b5[k0:k1])
            else:
                b = bf[cls // 2]
                wp[cls, 2] = -(b / 25.0) * b5
                wp[cls, 3] = b * idm - (b / 25.0) * b5
        wpf = np.ascontiguousarray(
            wp.transpose(2, 0, 1, 3).reshape(128, 5 * 4 * 128))

        # per-partition scalars; slot 2 = merged tile (img0 rows at psum
        # partitions 0..32, img1 rows at 64..96)
        t2m = np.zeros((IMGS_PER_CORE + 1, 128, 1), np.float32)
        ftm = np.zeros((IMGS_PER_CORE + 1, 128, 1), np.float32)
        for i in range(IMGS_PER_CORE):
            t2m[i, :, 0] = (0.5 * es[ii[i]]) ** 2
            ftm[i, :, 0] = fts[ii[i]]
        t2m[2, 0:64, 0] = (0.5 * es[ii[0]]) ** 2
        t2m[2, 64:128, 0] = (0.5 * es[ii[1]]) ** 2
        ftm[2, 0:64, 0] = fts[ii[0]]
        ftm[2, 64:128, 0] = fts[ii[1]]

        maps.append({
            "x": np.ascontiguousarray(mask[ii]),
            "wp": wpf,
            "thr2": t2m,
            "ftd": ftm,
        })
    return maps


def kernel(mask, blur_strength, edge_sensitivity, final_threshold):
    global _compiled, last_results
    if _compiled is None:
        _compiled = _build()
    maps = _in_maps(mask, blur_strength, edge_sensitivity, final_threshold)
    res = run_bass_kernel_spmd(_compiled, maps, core_ids=list(range(N_CORES)))
    last_results = res
    out = np.empty((16, 1, H, W), np.float32)
    for c in range(N_CORES):
        out[2 * c:2 * c + 2, 0] = res.results[c]["out"]
    return out
